# revision 10
# baseline (speedup 1.0000x reference)
# MoE top-2 routing kernel for 8 Trainium2 NeuronCores (paired expert-
# parallel).
#
# Problem (hardcoded shapes): T=2048 tokens, D=2048 model dim, F=4096 ffn dim,
# E=8 experts, top-2 routing with renormalized softmax weights.
#
# Sharding: experts are paired (heaviest with lightest by routed-token
# count); each pair of experts is split 50/50 across a pair of cores, so
# every core hosts two expert slots: slot A (half of a heavy expert, nA
# columns) and slot B (half of a light expert, nB columns). This cuts the
# per-core token capacity from max_e count_e (536 here) to
# ceil(max_heavy/2) + ceil(max_light/2) (520 here) at the cost of
# streaming two experts' weights per core (~100 MB bf16, ~293 GB/s demand
# vs ~360 GB/s measured single-ring DMA capability).
#
# The host does dispatch/data placement only: an fp32 router pre-pass
# picks each token's top-2 experts (selection gap >> fp32 noise), computes
# renormalized top-2 softmax weights in float64, gathers/transposes each
# core's tokens (slot A cols then slot B cols), and pre-tiles the weights
# so every device DMA is a single fat contiguous block.
#
# Device layout is fully weight-stationary, tokens always moving in
# columns: g[f,t] = sum_d wg[d,f] x[d,t] (lhsT = 128x128 wg tile, rhs =
# xT cols); h = silu(g)*u lands directly in [f,t] layout so the down
# matmul y[d,t] = sum_f wd[f,d] h[f,t] needs no PE transposes. Per-token
# router weights are a host-provided [128, C] broadcast applied inside the
# PSUM->SBUF copy of y. Output is yT [D, C]; the host scatter-adds slot
# slices back into [T, D].
#
# Startup: ~5us of throwaway matmuls warm the PE clock-gate (HAM) while
# the startup DMAs land; the first fi's weights are split into per-expert
# DMAs and interleaved with the x groups at the head of the sync HWDGE
# ring (FIFO) to minimize time-to-first-matmul. (The scalar HWDGE ring
# measured ~2x slower than sync; everything latency-critical stays on
# sync, y stores go to the gpsimd SWDGE ring.)

import os
import numpy as np
import ml_dtypes

_BF16NP = ml_dtypes.bfloat16

import concourse.bass as bass
import concourse.bacc as bacc
import concourse.mybir as mybir
import concourse.tile as tile
from concourse import bass_utils

FP32 = mybir.dt.float32
BF16 = mybir.dt.bfloat16
ACTF = mybir.ActivationFunctionType

T, D, F, E = 2048, 2048, 4096, 8
NCORES = 8
ND = D // 128    # 16 d-tiles
NF = F // 128    # 32 f-tiles


def _chunks_for(c0, C):
    """Split C token columns (starting at c0) into PSUM-bank chunks."""
    nch = (C + 511) // 512
    out, rem, p = [], C, c0
    for i in range(nch):
        cn = -(-(rem // (nch - i)) // 4) * 4
        cn = min(cn, rem)
        out.append((p, cn))
        p += cn
        rem -= cn
    return out


def build_program(nA, nB):
    C = nA + nB
    # chunk lists per slot: (col_offset, n_cols, slot_index)
    slots = ([(c0, cn, 0) for (c0, cn) in _chunks_for(0, nA)]
             + [(c0, cn, 1) for (c0, cn) in _chunks_for(nA, nB)])
    nc = bacc.Bacc(
        "TRN2",
        target_bir_lowering=False,
        debug=False,
        enable_asserts=False,
        num_devices=NCORES,
    )
    # x in [p, d, t] tile layout [128, 16*C]: row p, col d*C+t holds
    # xT[d*128+p, t] (slot A tokens then slot B tokens per d-tile)
    x_d = nc.dram_tensor("x", [128, ND * C], BF16, kind="ExternalInput").ap()
    # router weight per token, broadcast to [128, C] on host, fp32
    wb_d = nc.dram_tensor("wb", [128, C], FP32, kind="ExternalInput").ap()
    # fi=0 per-expert weight blocks (startup-critical, small DMAs):
    #   rows p, cols [A: d*128+q | B: d*128+q]
    # fi>=1 fused pair blocks: row fi*128+p, cols [gA | gB | uA | uB]
    w0g_d = nc.dram_tensor("w0g", [128, 2 * D], BF16, kind="ExternalInput").ap()
    w0u_d = nc.dram_tensor("w0u", [128, 2 * D], BF16, kind="ExternalInput").ap()
    wgu_d = nc.dram_tensor("wgu", [F, 4 * D], BF16, kind="ExternalInput").ap()
    # wd pair blocks: row dt*128+p, cols [A: fi*128+q | B: fi*128+q]
    wd_d = nc.dram_tensor("wd", [D, 2 * F], BF16, kind="ExternalInput").ap()
    # output yT [D, C] fp32
    y_d = nc.dram_tensor("y", [D, C], FP32, kind="ExternalOutput").ap()

    with tile.TileContext(nc) as tc:
        with (
            tc.tile_pool(name="const", bufs=1) as const_pool,
            tc.tile_pool(name="xp", bufs=1) as x_pool,
            tc.tile_pool(name="hp", bufs=1) as h_pool,
            tc.tile_pool(name="w0", bufs=1) as w0_pool,
            tc.tile_pool(name="wgu", bufs=3) as wgu_pool,
            tc.tile_pool(name="wdp", bufs=3) as wd_pool,
            tc.tile_pool(name="yp", bufs=4) as y_pool,
            tc.tile_pool(name="stp", bufs=4) as st_pool,
            tc.tile_pool(name="ps", bufs=8, space="PSUM") as ps_pool,
        ):
            # ---- PE warmup: ~4us of throwaway matmuls so the HAM
            # clock-gate opens to 8/8 while the startup DMAs land ----
            dum = const_pool.tile([128, 160], BF16, tag="dum", name="dum")
            nc.vector.memset(dum[:], 0.0)
            pdum = ps_pool.tile([128, 512], FP32, tag="ps", name="ps")
            for _ in range(42):
                nc.tensor.matmul(pdum[:, :160], dum[:, :128], dum[:],
                                 start=True, stop=True)

            # startup-critical DMA order on the sync ring (FIFO): fi=0
            # gate-A block, x groups, fi=0 gate-B, up blocks, wb.
            w0gA = w0_pool.tile([128, D], BF16, tag="w0gA", name="w0gA")
            nc.sync.dma_start(w0gA[:], w0g_d[:, 0:D])
            xt = []
            for g in range(ND // 4):
                xg = x_pool.tile([128, 4 * C], BF16, tag=f"x{g}", name=f"x{g}")
                nc.sync.dma_start(xg[:], x_d[:, g * 4 * C:(g + 1) * 4 * C])
                xt.append(xg)
            w0gB = w0_pool.tile([128, D], BF16, tag="w0gB", name="w0gB")
            nc.sync.dma_start(w0gB[:], w0g_d[:, D:2 * D])
            w0uA = w0_pool.tile([128, D], BF16, tag="w0uA", name="w0uA")
            nc.sync.dma_start(w0uA[:], w0u_d[:, 0:D])
            w0uB = w0_pool.tile([128, D], BF16, tag="w0uB", name="w0uB")
            nc.sync.dma_start(w0uB[:], w0u_d[:, D:2 * D])
            wb_sb = const_pool.tile([128, C], FP32, tag="wb", name="wb_sb")
            nc.sync.dma_start(wb_sb[:], wb_d[:])

            def xs(d, c0, cn):
                return xt[d // 4][:, (d % 4) * C + c0:(d % 4) * C + c0 + cn]

            # ---- phase 1: gate/up matmuls + silu*up -> h[f, t] ----
            hs = []
            for fi in range(NF):
                if fi == 0:
                    wslc = [w0gA[:], w0gB[:], w0uA[:], w0uB[:]]
                else:
                    wgu = wgu_pool.tile([128, 4 * D], BF16, tag="w", name="wgu")
                    nc.sync.dma_start(wgu[:], wgu_d[fi * 128:(fi + 1) * 128, :])
                    wslc = [wgu[:, i * D:(i + 1) * D] for i in range(4)]
                pg = [ps_pool.tile([128, 512], FP32, tag="ps", name="ps")
                      for _ in slots]
                pu = [ps_pool.tile([128, 512], FP32, tag="ps", name="ps")
                      for _ in slots]
                for d in range(ND):
                    for ci, (c0, cn, sl) in enumerate(slots):
                        nc.tensor.matmul(
                            pg[ci][:, :cn], wslc[sl][:, d * 128:(d + 1) * 128],
                            xs(d, c0, cn),
                            start=(d == 0), stop=(d == ND - 1),
                        )
                for d in range(ND):
                    for ci, (c0, cn, sl) in enumerate(slots):
                        nc.tensor.matmul(
                            pu[ci][:, :cn],
                            wslc[2 + sl][:, d * 128:(d + 1) * 128],
                            xs(d, c0, cn),
                            start=(d == 0), stop=(d == ND - 1),
                        )
                h = h_pool.tile([128, C], BF16, tag=f"h{fi}", name=f"h{fi}")
                for ci, (c0, cn, sl) in enumerate(slots):
                    st = st_pool.tile([128, 512], FP32, tag="st", name="st")
                    nc.scalar.activation(st[:, :cn], pg[ci][:, :cn], ACTF.Silu)
                    nc.vector.tensor_mul(h[:, c0:c0 + cn], st[:, :cn],
                                         pu[ci][:, :cn])
                hs.append(h)

            # ---- phase 2: down matmuls, router-weight scale, store yT ----
            for dt in range(ND):
                wdt = wd_pool.tile([128, 2 * F], BF16, tag="wd", name="wdt")
                nc.sync.dma_start(wdt[:], wd_d[dt * 128:(dt + 1) * 128, :])
                # chunk-outer: earlier chunks' scale+store overlap later
                # chunks' MMs, so only the last store is exposed at the tail
                for (c0, cn, sl) in slots:
                    py = ps_pool.tile([128, 512], FP32, tag="ps", name="ps")
                    for fi in range(NF):
                        nc.tensor.matmul(
                            py[:, :cn],
                            wdt[:, sl * F + fi * 128:sl * F + (fi + 1) * 128],
                            hs[fi][:, c0:c0 + cn],
                            start=(fi == 0), stop=(fi == NF - 1),
                        )
                    ysb = y_pool.tile([128, 512], FP32, tag="y", name="ysb")
                    nc.vector.tensor_mul(ysb[:, :cn], py[:, :cn],
                                         wb_sb[:, c0:c0 + cn])
                    nc.gpsimd.dma_start(
                        y_d[dt * 128:(dt + 1) * 128, c0:c0 + cn], ysb[:, :cn])

    nc.compile()
    return nc


_PROGRAM_CACHE = {}


def _get_program(nA, nB):
    key = (nA, nB)
    if key not in _PROGRAM_CACHE:
        _PROGRAM_CACHE[key] = build_program(nA, nB)
    return _PROGRAM_CACHE[key]


def _route_host(x_TD, router_w):
    """Host dispatch: top-2 ids + renormalized top-2 softmax weights."""
    logits = (x_TD @ router_w).astype(np.float64)  # selection gap >> fp32 err
    order = np.argsort(-logits, axis=1, kind="stable")
    top2 = order[:, :2]
    z = logits - logits.max(axis=1, keepdims=True)
    p = np.exp(z)
    p /= p.sum(axis=1, keepdims=True)
    pw = np.take_along_axis(p, top2, axis=1)       # [T, 2]
    pw /= pw.sum(axis=1, keepdims=True)
    return top2, pw


def _retile_wgu(w):
    """[D, F] -> [F, D] rows fi*128+p, cols d*128+q, bf16 (lhsT tiles)."""
    m = w.astype(_BF16NP).reshape(ND, 128, NF, 128).transpose(2, 1, 0, 3)
    return np.ascontiguousarray(m).reshape(F, D)


def _retile_wd(w):
    """[F, D] -> [D, F] rows dt*128+p, cols fi*128+q, bf16 (lhsT tiles)."""
    m = w.astype(_BF16NP).reshape(NF, 128, ND, 128).transpose(2, 1, 0, 3)
    return np.ascontiguousarray(m).reshape(D, F)


def kernel_with_results(x_TD, router_w, w_gate, w_up, w_down):
    x_TD = np.ascontiguousarray(x_TD, np.float32)
    router_w = np.ascontiguousarray(router_w, np.float32)
    w_gate = np.ascontiguousarray(w_gate, np.float32)
    w_up = np.ascontiguousarray(w_up, np.float32)
    w_down = np.ascontiguousarray(w_down, np.float32)

    top2, pw = _route_host(x_TD, router_w)
    idx_lists = [np.where((top2 == e).any(axis=1))[0] for e in range(E)]
    counts = [len(ix) for ix in idx_lists]
    # pair heaviest with lightest; split each expert 50/50 over its 2 cores
    order = np.argsort(-np.asarray(counts), kind="stable")
    heavy, light = order[:E // 2], order[E // 2:][::-1]
    nA = max(64, -(-max(-(-counts[e] // 2) for e in heavy) // 4) * 4)
    nB = max(64, -(-max(-(-counts[e] // 2) for e in light) // 4) * 4)
    C = nA + nB

    nc = _get_program(nA, nB)

    wgt = {int(e): _retile_wgu(w_gate[e]) for e in order}
    wut = {int(e): _retile_wgu(w_up[e]) for e in order}
    wdt = {int(e): _retile_wd(w_down[e]) for e in order}

    # core assignment: pair i -> cores (2i, 2i+1); fragments[(core)] =
    # [(expert, token_idx_frag, col_offset), ...] for slots A and B
    frags = [[] for _ in range(NCORES)]
    in_maps = [None] * NCORES
    for i in range(E // 2):
        eA, eB = int(heavy[i]), int(light[i])
        ixA, ixB = idx_lists[eA], idx_lists[eB]
        hA = (len(ixA) + 1) // 2
        hB = (len(ixB) + 1) // 2
        wgu_pair = np.concatenate(
            [wgt[eA], wgt[eB], wut[eA], wut[eB]], axis=1)  # [F, 4D]
        wd_pair = np.concatenate([wdt[eA], wdt[eB]], axis=1)  # [D, 2F]
        for half in range(2):
            core = 2 * i + half
            fA = ixA[:hA] if half == 0 else ixA[hA:]
            fB = ixB[:hB] if half == 0 else ixB[hB:]
            frags[core] = [(eA, fA, 0), (eB, fB, nA)]
            xg = np.zeros((C, D), np.float32)
            xg[:len(fA)] = x_TD[fA]
            xg[nA:nA + len(fB)] = x_TD[fB]
            xTe = np.ascontiguousarray(
                xg.T.astype(_BF16NP).reshape(ND, 128, C).transpose(1, 0, 2)
            ).reshape(128, ND * C)
            wtok = np.zeros((C,), np.float32)
            for e, fx, c0 in frags[core]:
                sel = (top2[fx] == e).argmax(axis=1)
                wtok[c0:c0 + len(fx)] = pw[fx, sel]
            wb = np.ascontiguousarray(
                np.broadcast_to(wtok[None, :], (128, C)), np.float32)
            in_maps[core] = {
                "x": xTe,
                "wb": wb,
                "w0g": np.ascontiguousarray(wgu_pair[0:128, :2 * D]),
                "w0u": np.ascontiguousarray(wgu_pair[0:128, 2 * D:]),
                "wgu": wgu_pair,
                "wd": wd_pair,
            }

    try:
        res = bass_utils.run_bass_kernel_spmd(
            nc, in_maps, core_ids=list(range(NCORES))
        )
    except ModuleNotFoundError:
        # Tracing requested via env but the axon NTFF hook module is absent
        # in this image — rerun without tracing.
        os.environ["BASS_NEVER_TRACE"] = "1"
        res = bass_utils.run_bass_kernel_spmd(
            nc, in_maps, core_ids=list(range(NCORES))
        )

    out = np.zeros((T, D), np.float32)
    for core in range(NCORES):
        y = res.results[core]["y"]  # [D, C]
        for e, fx, c0 in frags[core]:
            out[fx] += y[:, c0:c0 + len(fx)].T
    return out, res


def kernel(**inputs):
    out, _ = kernel_with_results(**inputs)
    return out
